# revision 2
# baseline (speedup 1.0000x reference)
import numpy as np
import jax
import jax.numpy as jnp

NQ, NL, SD = 8, 3, 256
EPS_BN = 1e-5
NDEV = 8


def _circuit_matrix(qw):
    """Simulate the VQC on the 256 basis states (numpy, complex128).
    Returns PSI with PSI[b, :] = circuit(e_b); for a batch X of real
    amplitude vectors the final state is X @ PSI (circuit is linear)."""
    qw = np.asarray(qw, np.float64)
    psi = np.eye(SD, dtype=np.complex128).reshape((SD,) + (2,) * NQ)
    for l in range(NL):
        for q in range(NQ):
            phi, theta, omega = qw[l, q]
            c, s = np.cos(theta / 2.0), np.sin(theta / 2.0)
            U = np.array([
                [np.exp(-0.5j * (phi + omega)) * c,
                 -np.exp(0.5j * (phi - omega)) * s],
                [np.exp(0.5j * (phi - omega)) * s,
                 np.exp(0.5j * (phi + omega)) * c],
            ])
            psi = np.moveaxis(psi, q + 1, -1)
            psi = psi @ U.T
            psi = np.moveaxis(psi, -1, q + 1)
        for q in range(NQ):
            c_, t_ = q, (q + 1) % NQ
            ca, ta = c_ + 1, t_ + 1
            idx = (slice(None),) * ca
            s0 = psi[idx + (0,)]
            s1 = np.flip(psi[idx + (1,)], axis=ta - 1 if ta > ca else ta)
            psi = np.stack([s0, s1], axis=ca)
    return psi.reshape(SD, SD)


def kernel(flux, scalars, conv1_w, bn1_g, bn1_b, conv2_w, bn2_g, bn2_b,
           proj_w1, proj_b1, proj_w2, proj_b2, q_weights,
           head_w1, head_b1, head_bn_g, head_bn_b, head_w2, head_b2):
    B = flux.shape[0]

    PSI = _circuit_matrix(q_weights)
    PSIr = jnp.asarray(np.ascontiguousarray(PSI.real).astype(np.float32))
    PSIi = jnp.asarray(np.ascontiguousarray(PSI.imag).astype(np.float32))
    bits = (np.arange(SD)[None, :] >> (NQ - 1 - np.arange(NQ))[:, None]) & 1
    signsT = jnp.asarray((1.0 - 2.0 * bits).T.astype(np.float32))  # [SD, NQ]

    bn_s = np.float32(1.0 / np.sqrt(1.0 + EPS_BN))
    s1 = jnp.asarray(np.asarray(bn1_g) * bn_s)
    b1 = jnp.asarray(bn1_b)
    s2 = jnp.asarray(np.asarray(bn2_g) * bn_s)
    b2 = jnp.asarray(bn2_b)
    sh = jnp.asarray(np.asarray(head_bn_g) * bn_s)
    bh = jnp.asarray(head_bn_b)

    w1 = jnp.asarray(conv1_w)
    w2 = jnp.asarray(conv2_w)
    pw1T = jnp.asarray(np.asarray(proj_w1).T)
    pb1 = jnp.asarray(proj_b1)
    pw2T = jnp.asarray(np.asarray(proj_w2).T)
    pb2 = jnp.asarray(proj_b2)
    hw1T = jnp.asarray(np.asarray(head_w1).T)
    hb1 = jnp.asarray(head_b1)
    hw2T = jnp.asarray(np.asarray(head_w2).T)
    hb2 = jnp.asarray(head_b2)
    uniform = jnp.full((SD,), 1.0 / np.sqrt(SD), jnp.float32)

    def fwd(fx, sc):
        h = jax.lax.conv_general_dilated(
            fx, w1, (4,), [(7, 7)], dimension_numbers=('NCH', 'OIH', 'NCH'))
        h = jax.nn.relu(h * s1[None, :, None] + b1[None, :, None])
        Bs, C, L = h.shape
        h = h[..., :(L // 4) * 4].reshape(Bs, C, L // 4, 4).max(-1)
        h = jax.lax.conv_general_dilated(
            h, w2, (2,), [(3, 3)], dimension_numbers=('NCH', 'OIH', 'NCH'))
        h = jax.nn.relu(h * s2[None, :, None] + b2[None, :, None])
        L2 = h.shape[-1]
        cols = [h[..., (i * L2) // 8: -(-((i + 1) * L2) // 8)].mean(-1)
                for i in range(8)]
        h = jnp.stack(cols, axis=-1).reshape(Bs, -1)
        h = jax.nn.relu(h @ pw1T + pb1)
        x = h @ pw2T + pb2
        n = jnp.linalg.norm(x, axis=-1, keepdims=True)
        x = x / jnp.maximum(n, 1e-12)
        n2 = jnp.linalg.norm(x, axis=-1, keepdims=True)
        x = jnp.where(n2 < 1e-8, uniform[None, :], x)
        re = x @ PSIr
        im = x @ PSIi
        probs = re * re + im * im
        q = probs @ signsT
        hh = jnp.concatenate([q, sc], axis=1)
        hh = jax.nn.relu((hh @ hw1T + hb1) * sh[None, :] + bh[None, :])
        return hh @ hw2T + hb2

    pf = jax.pmap(fwd)
    fx_sh = np.asarray(flux, np.float32).reshape(NDEV, B // NDEV, 1, flux.shape[-1])
    sc_sh = np.asarray(scalars, np.float32).reshape(NDEV, B // NDEV, scalars.shape[-1])
    out = pf(fx_sh, sc_sh)
    return np.asarray(out, np.float32).reshape(B, 3)
